# revision 40
# baseline (speedup 1.0000x reference)
"""Multi-head attention (B=2, S=2048, E=1024, H=16, D=64) on 8 trn2 cores.

Sharding: core c = (b, g) with b = c // 4 (batch), g = c % 4 (head group of
4 heads = 256 features). Each core computes Q/K/V projections for its head
group, full attention for its 4 heads, and a partial output projection
(columns of its group); a ReduceScatter over the 4 cores of each batch sums
the partials and leaves each core with a [512, 1024] slice of the final
output. The host concatenates the slices and adds bo.

Device-side layouts (host pre-transposes/casts):
  xT  [1024, 2048]  query[b].T                 (compute dtype)
  wqT/wkT/wvT [1024, 256]  W[g*256:(g+1)*256, :].T
  woT [256, 1024]          Wo[:, g*256:(g+1)*256].T
  bkq_c [128, 4]           K/Q bias columns (fp32, added in PSUM->SBUF copy)
  bv_b [128, 256]          V bias row pre-broadcast over partitions

On-chip dataflow per core (all contractions on the partition dim):
  Q^T,K^T [f,s] = (W^T chunk).T @ x^T + bias   (bias via DVE tensor_scalar)
  V [k,f] = (x^T chunk).T @ W^T + bias         (bias via DVE tensor_tensor)
  S^T [k,q] = (K^T chunk).T @ Q^T   (K = d = 64)
  P^T = exp(S^T / 8)  via ScalarE, PSUM -> SBUF, cast to compute dtype
  O  [q,d+1] = (P^T chunk).T @ V'   with V' = [V | 1] (col d = denom)
    -- flipped AV: stationary = P^T chunk, so the matmul's free dim is
       d+1 = 65 instead of 512, quartering tensor-engine time there.
  O <- O * (1/denom)  (DVE per-partition scalar multiply), then
  O^T via DMA crossbar transpose back into SBUF (idle DMA engines)
  Y [s,f] = (O^T chunk).T @ Wo^T  -> ReduceScatter(+) over the 4-core group
  (Y is written bf16; the host accumulates/adds bo in fp32.)

Scheduling: engine queues are in-order, so emission order is everything.
Dummy matmuls warm the PE p-state during the input-DMA wait; x arrives in
column groups and the K projection is split into half-tiles emitted just
ahead of the score group that first needs each one, so ScalarE starts
~10us in and then runs nearly stall-free; every iteration interleaves the
previous q-chunk's AV (split per 128-query subblock) between score groups
with a half-block skew that keeps each exp window's tensor-engine load
under budget; the last q-chunk's AV rides inside the final iteration so
the tail owes only one head + transposes + out-projection.
"""

import numpy as np

B, S, E, H, D = 2, 2048, 1024, 16, 64
G = 4            # head groups (tensor-parallel)
GH = H // G      # heads per group = 4
GF = GH * D      # features per group = 256
NC = 8
SCALE = 1.0 / np.sqrt(D)

_CACHE = {}


def _build(mode: str, collective: bool, reps: int = 1):
    import concourse.bass as bass
    import concourse.mybir as mybir
    import concourse.tile as tile
    from concourse import bacc

    dt = mybir.dt
    C = {"bf16": dt.bfloat16, "f32r": dt.float32r, "fp32": dt.float32}[mode]
    f32 = dt.float32

    nc = bacc.Bacc()

    xT = nc.dram_tensor("xT", [E, S], C, kind="ExternalInput")
    wqT = nc.dram_tensor("wqT", [E, GF], C, kind="ExternalInput")
    wkT = nc.dram_tensor("wkT", [E, GF], C, kind="ExternalInput")
    wvT = nc.dram_tensor("wvT", [E, GF], C, kind="ExternalInput")
    woT = nc.dram_tensor("woT", [GF, E], C, kind="ExternalInput")
    bkq_c = nc.dram_tensor("bkq_c", [128, 4], f32, kind="ExternalInput")
    bv_b = nc.dram_tensor("bv_b", [128, GF], C, kind="ExternalInput")
    # bf16 output halves the output DMA time; the host accumulates in fp32
    if collective:
        yout = nc.dram_tensor("yout", [S // G, E], C, kind="ExternalOutput")
    else:
        yout = nc.dram_tensor("yout", [S, E], C, kind="ExternalOutput")

    EC = E // 128    # 8 e-chunks
    QC = S // 512    # 4 q-chunks
    KB = S // 128    # 16 k-blocks
    VW = GH * (D + 1)  # 260: V' row width (per head: 64 data + 1 ones col)

    def emit_body(nc, tc, res, do_coll):
        # x as one resident tile, e-chunk major; column-group DMAs fill it
        xAll = res.tile([128, EC * S], C, tag="xAll", name="xAll")

        def xs(ec, c0, c1):
            return xAll[:, ec * S + c0:ec * S + c1]

        wqT_sb = res.tile([128, EC * GF], C, tag="wqT")
        wkT_sb = res.tile([128, EC * GF], C, tag="wkT")
        wvT_sb = res.tile([128, EC * GF], C, tag="wvT")
        woT_sb = res.tile([128, 2 * E], C, tag="woT")
        QT_sb = [[res.tile([128, 512], C, tag=f"QT{fb}_{qc}", name=f"QT{fb}_{qc}")
                  for qc in range(QC)] for fb in range(2)]
        KT_sb = [[res.tile([128, 512], C, tag=f"KT{fb}_{qc}", name=f"KT{fb}_{qc}")
                  for qc in range(QC)] for fb in range(2)]
        V_sb = [res.tile([128, VW], C, tag=f"V{kb}", name=f"V{kb}")
                for kb in range(KB)]
        # O^T per qc: [128, 2*512]: free = hb*512 + q  (hb = head-pair block)
        OT2_sb = [res.tile([128, 2 * 512], C, tag=f"OT{qc}", name=f"OT{qc}")
                  for qc in range(QC)]
        bkq_sb = res.tile([128, 4], f32, tag="bkq")
        bk_sb = bkq_sb[:, 0:2]
        bq_sb = bkq_sb[:, 2:4]
        bv_sb = res.tile([128, GF], C, tag="bv")

        # input DMAs, ordered for the critical path: wk, x cols 0:512, wq,
        # qk biases, remaining x columns, then v/o weights and constants.
        def dma_w(dst, src, nech):
            nc.sync.dma_start(
                out=dst[:].rearrange("p (g f) -> p g f", g=nech),
                in_=src[:].rearrange("(g p) f -> p g f", p=128))

        def dma_xcols(c0, c1):
            nc.sync.dma_start(
                out=xAll[:].rearrange("p (g s) -> p g s", g=EC)[:, :, c0:c1],
                in_=xT[:].rearrange("(g p) s -> p g s", p=128)[:, :, c0:c1])

        dma_w(wkT_sb, wkT, EC)
        dma_xcols(0, 256)
        dma_xcols(256, 512)
        nc.sync.dma_start(out=bkq_sb[:], in_=bkq_c[:])
        dma_w(wqT_sb, wqT, EC)
        for qp in range(1, 4):
            dma_xcols(qp * 512, qp * 512 + 512)
        nc.sync.dma_start(out=bv_sb[:], in_=bv_b[:])
        dma_w(wvT_sb, wvT, EC)
        dma_w(woT_sb, woT, 2)

        # ones columns of V'
        scrap = res.tile([1, 128], C, tag="scrap")
        nc.gpsimd.memset(scrap[:], 0.0)
        for kb in range(KB):
            nc.gpsimd.memset(
                V_sb[kb][:].rearrange("p (h x) -> p h x", x=D + 1)[:, :, D:D + 1],
                1.0)

        # ---- emit helpers ----
        def emit_qk_proj(pp, w_sb, dst, b_sb, fb, qc, c0=0, c1=512):
            # Q^T / K^T group in [f, s] layout: stationary = W^T chunk
            w = c1 - c0
            pq = pp.tile([128, 512], f32, tag="pq", name="pq", bufs=2)
            for ec in range(EC):
                nc.tensor.matmul(
                    pq[:, 0:w],
                    lhsT=w_sb[:, ec * GF + fb * 128:ec * GF + fb * 128 + 128],
                    rhs=xs(ec, qc * 512 + c0, qc * 512 + c1),
                    start=(ec == 0), stop=(ec == EC - 1))
            # bias folded into the PSUM->SBUF copy as a per-partition scalar
            nc.vector.tensor_scalar_add(
                out=dst[fb][qc][:, c0:c1], in0=pq[:, 0:w],
                scalar1=b_sb[:, fb:fb + 1])

        def emit_v_proj_pair(pp, j):
            # V group in natural [k, f] layout for k-blocks 2j, 2j+1
            pv = pp.tile([128, 512], f32, tag="pav", name="pv", bufs=2)
            for t in range(2):
                kb = 2 * j + t
                for ec in range(EC):
                    nc.tensor.matmul(
                        pv[:, t * GF:(t + 1) * GF],
                        lhsT=xs(ec, kb * 128, kb * 128 + 128),
                        rhs=wvT_sb[:, ec * GF:(ec + 1) * GF],
                        start=(ec == 0), stop=(ec == EC - 1))
            for t in range(2):
                kb = 2 * j + t
                # bias via host-broadcast row, fused into the PSUM->SBUF copy
                nc.vector.tensor_tensor(
                    out=V_sb[kb][:].rearrange(
                        "p (h x) -> p h x", x=D + 1)[:, :, 0:D],
                    in0=pv[:, t * GF:(t + 1) * GF].rearrange(
                        "p (h d) -> p h d", d=D),
                    in1=bv_sb[:].rearrange("p (h d) -> p h d", d=D),
                    op=mybir.AluOpType.add)

        def emit_score_group(ps, h, qc, ptt, gi):
            hb, hr = h // 2, (h % 2) * D
            kb0 = 2 * gi
            pst = ps.tile([128, 2 * 512], f32, tag="pst", name="pst", bufs=2)
            for kj in range(2):
                kb = kb0 + kj
                nc.tensor.matmul(
                    pst[:, kj * 512:(kj + 1) * 512],
                    lhsT=KT_sb[hb][kb // 4][hr:hr + D,
                                            (kb % 4) * 128:(kb % 4) * 128 + 128],
                    rhs=QT_sb[hb][qc][hr:hr + D, :],
                    start=True, stop=True)
            nc.scalar.activation(
                ptt[:, kb0 * 512:(kb0 + 2) * 512],
                pst[:],
                mybir.ActivationFunctionType.Exp, scale=SCALE)

        def emit_av_qs(pav, recp, O2, h, ptt, qs):
            # Flipped AV for one 128-query subblock: stationary = P^T chunk
            # [128k, 128q], moving = V' [128k, 65]; accumulate over k-blocks.
            for kb in range(KB):
                nc.tensor.matmul(
                    pav[:, qs * (D + 1):(qs + 1) * (D + 1)],
                    lhsT=ptt[:, kb * 512 + qs * 128:kb * 512 + qs * 128 + 128],
                    rhs=V_sb[kb][:, h * (D + 1):(h + 1) * (D + 1)],
                    start=(kb == 0), stop=(kb == KB - 1))
            rec = recp.tile([128, 1], f32, tag="rec", name="rec")
            nc.vector.reciprocal(
                rec[:], pav[:, qs * (D + 1) + D:qs * (D + 1) + D + 1])
            nc.vector.tensor_scalar_mul(
                out=O2[qs][:, h * D:(h + 1) * D],
                in0=pav[:, qs * (D + 1):qs * (D + 1) + D],
                scalar1=rec[:])

        def emit_transposes(pp, O2, qc, hb, eng=None):
            # O [q, f-pair] -> O^T [f-pair, q] for head pair hb, all 4 qs,
            # via the DMA crossbar transpose (idle DMA engines, no PE/DVE)
            eng = eng or nc.sync
            for qs in range(4):
                eng.dma_start_transpose(
                    out=OT2_sb[qc][:, hb * 512 + qs * 128:
                                   hb * 512 + qs * 128 + 128],
                    in_=O2[qs][:, hb * 128:hb * 128 + 128])

        def emit_outproj_sb(po, ysb, sb, act_copy=False):
            qc = sb // 4
            for fc in range(2):
                pyt = po.tile([128, 512], f32, tag=("pav" if fc == 0 else "pq"),
                              name="pyt")
                for ec in range(2):
                    nc.tensor.matmul(
                        pyt[:],
                        lhsT=OT2_sb[qc][:, ec * 512 + (sb % 4) * 128:
                                        ec * 512 + (sb % 4) * 128 + 128],
                        rhs=woT_sb[:, ec * E + fc * 512:ec * E + fc * 512 + 512],
                        start=(ec == 0), stop=(ec == 1))
                yt = ysb.tile([128, 512], C, tag="yt", name="yt")
                if act_copy and fc == 1:
                    # tail only: ScalarE is idle once the exps are done
                    nc.scalar.copy(yt[:], pyt[:])
                else:
                    nc.vector.tensor_copy(yt[:], pyt[:])
                dst = y_part if collective else yout
                nc.sync.dma_start(
                    out=dst[sb * 128:(sb + 1) * 128, fc * 512:(fc + 1) * 512],
                    in_=yt[:])

        # ---- emission (order = scheduler priority; engine queues are
        # in-order, so every dependency must appear before its consumer,
        # and slow-to-unblock work must not be emitted ahead of urgent
        # work on the same engine) ----
        # PSUM (8 banks): "pst" [128,1024] x2 = 4 banks (scores+exp),
        # "pq" [128,512] x2 = 2 banks (projections, pT staging, outproj),
        # "pav" [128,512] x2 = 2 banks (V-proj, AV accum, outproj).
        with tc.tile_pool(name="dram", bufs=1, space="DRAM") as dram, \
             tc.tile_pool(name="pall", bufs=2, space="PSUM") as pall, \
             tc.tile_pool(name="ptp", bufs=6) as ptp, \
             tc.tile_pool(name="o2p", bufs=2) as o2p, \
             tc.tile_pool(name="recp", bufs=4) as recp, \
             tc.tile_pool(name="ysb", bufs=4) as ysb:
            if collective:
                y_part = dram.tile([S, E], C, tag="ypart")
                rs_out = dram.tile([S // G, E], C, tag="rsout")

            def new_ptt(h):
                return ptp.tile([128, KB * 512], C, tag="ptt", name=f"ptt{h}")

            def exp_block(h, qc, ptt, extras):
                """Emit the 8 score groups + exp calls for (h, qc), with
                `extras` (list of thunks) interleaved between groups."""
                for gi in range(8):
                    emit_score_group(pall, h, qc, ptt, gi)
                    if gi >= 1 and extras:
                        extras.pop(0)()
                while extras:
                    extras.pop(0)()

            # --- q-chunk 0 phase: K/Q projections interleaved with the
            # first heads' score groups so ScalarE starts ASAP.
            def kp(fb, kc, half):
                return lambda: emit_qk_proj(pall, wkT_sb, KT_sb, bk_sb, fb, kc,
                                            half * 256, half * 256 + 256)

            ptts = {}
            # warm the tensor engine's p-state during the input-DMA wait:
            # back-to-back trivial matmuls keep it busy so the first real
            # projections run at full clock
            pwu = pall.tile([128, 2 * 512], f32, tag="pst", name="pwu", bufs=2)
            for _ in range(96):
                nc.tensor.matmul(pwu[0:1, 0:128], lhsT=scrap[:, 0:1],
                                 rhs=scrap[:], start=True, stop=True)
            nc.vector.tensor_copy(scrap[:], pwu[0:1, 0:128])
            # K projection in half-tiles, each emitted just ahead of the
            # score group that first needs it (in-order PE queue = JIT feed)
            kp(0, 0, 0)()
            emit_qk_proj(pall, wqT_sb, QT_sb, bq_sb, 0, 0)
            kp(0, 0, 1)()
            ptts[0] = new_ptt(0)
            exp_block(0, 0, ptts[0], [
                kp(0, 1, 0), kp(0, 1, 1), kp(0, 2, 0),
                kp(0, 2, 1), kp(0, 3, 0), kp(0, 3, 1)])
            ptts[1] = new_ptt(1)
            exp_block(1, 0, ptts[1], [
                lambda: emit_qk_proj(pall, wqT_sb, QT_sb, bq_sb, 1, 0),
                kp(1, 0, 0), kp(1, 0, 1), kp(1, 1, 0), kp(1, 1, 1)])
            ptts[2] = new_ptt(2)
            exp_block(2, 0, ptts[2], [
                kp(1, 2, 0), kp(1, 2, 1), kp(1, 3, 0), kp(1, 3, 1),
                lambda: emit_v_proj_pair(pall, 0)])
            ptts[3] = new_ptt(3)
            exp_block(3, 0, ptts[3], [
                lambda: emit_v_proj_pair(pall, 1),
                lambda: emit_qk_proj(pall, wqT_sb, QT_sb, bq_sb, 0, 1)])
            emit_qk_proj(pall, wqT_sb, QT_sb, bq_sb, 1, 1)

            # steady-state: exps of qc overlap AV of qc-1 (split per qs),
            # transposes of qc-1 complete within the iteration, outproj of
            # qc-2 rides along; the tail only owes qc3's AV + outproj.
            def av_extras(O2, h, ptt):
                # pav is allocated lazily at the first AV chunk so no other
                # same-tag allocation can slip between tile() and first write
                holder = {}

                def mk(qs):
                    def go():
                        if "pav" not in holder:
                            holder["pav"] = pall.tile(
                                [128, 512], f32, tag="pav", name="pav", bufs=2)
                        emit_av_qs(holder["pav"], recp, O2, h, ptt, qs)
                    return go
                return [mk(qs) for qs in range(4)]

            # Global half-block skew: in iteration qc, block h0 carries
            # AV(qc-2, h3) and blocks h1..h3 carry AV(qc-1, h0..h2), so
            # per-block tensor-engine load stays under the exp window.
            O2s = {}
            ptth = {0: ptts}
            for qc in range(1, QC):
                pqc = qc - 1
                O2s[pqc] = [o2p.tile([128, GH * D], C, tag=f"o2_{qs}",
                                     name="o2") for qs in range(4)]
                new_ptts = {}
                new_ptts[0] = new_ptt(0)
                if qc == 1:
                    ex = [lambda: emit_v_proj_pair(pall, 2),
                          lambda: emit_v_proj_pair(pall, 3),
                          lambda: emit_v_proj_pair(pall, 4),
                          lambda: emit_v_proj_pair(pall, 5)]
                else:
                    ex = av_extras(O2s[qc - 2], 3, ptth[qc - 2][3])
                exp_block(0, qc, new_ptts[0], ex)
                if qc >= 2:
                    emit_transposes(pall, O2s[qc - 2], qc - 2, 1)
                new_ptts[1] = new_ptt(1)
                ex = av_extras(O2s[pqc], 0, ptth[pqc][0])
                if qc == 1:
                    ex = [lambda: emit_v_proj_pair(pall, 6),
                          lambda: emit_v_proj_pair(pall, 7)] + ex
                exp_block(1, qc, new_ptts[1], ex)
                if qc >= 2:
                    for sb in range((qc - 2) * 4, (qc - 2) * 4 + 4):
                        emit_outproj_sb(pall, ysb, sb)
                new_ptts[2] = new_ptt(2)
                ex = av_extras(O2s[pqc], 1, ptth[pqc][1])
                if qc < QC - 1:
                    ex.append(lambda: emit_qk_proj(
                        pall, wqT_sb, QT_sb, bq_sb, 0, qc + 1))
                    ex.append(lambda: emit_qk_proj(
                        pall, wqT_sb, QT_sb, bq_sb, 1, qc + 1))
                else:
                    # last iteration: its own first heads' AV rides along so
                    # the tail only owes heads 2 and 3
                    O2s[qc] = [o2p.tile([128, GH * D], C, tag=f"o2_{qs}",
                                        name="o2") for qs in range(4)]
                    ex = ex + av_extras(O2s[qc], 0, new_ptts[0])
                exp_block(2, qc, new_ptts[2], ex)
                new_ptts[3] = new_ptt(3)
                ex = av_extras(O2s[pqc], 2, ptth[pqc][2])
                if qc == QC - 1:
                    ex = ex + av_extras(O2s[qc], 1, new_ptts[1])
                exp_block(3, qc, new_ptts[3], ex)
                emit_transposes(pall, O2s[pqc], pqc, 0)
                ptth[qc] = new_ptts
                ptts = new_ptts

            # tail: qc2's h3, then qc3's last heads + transposes + outproj
            q3, q2 = QC - 1, QC - 2
            for thunk in av_extras(O2s[q2], 3, ptth[q2][3]):
                thunk()
            emit_transposes(pall, O2s[q2], q2, 1)
            emit_transposes(pall, O2s[q3], q3, 0)
            for thunk in av_extras(O2s[q3], 2, ptth[q3][2]):
                thunk()
            for sb in range(q2 * 4, q2 * 4 + 4):
                emit_outproj_sb(pall, ysb, sb)
            for thunk in av_extras(O2s[q3], 3, ptth[q3][3]):
                thunk()
            emit_transposes(pall, O2s[q3], q3, 1)
            for sb in range(q3 * 4, q3 * 4 + 4):
                emit_outproj_sb(pall, ysb, sb, act_copy=True)

            if collective and do_coll:
                nc.gpsimd.collective_compute(
                    "ReduceScatter",
                    mybir.AluOpType.add,
                    replica_groups=[[0, 1, 2, 3], [4, 5, 6, 7]],
                    ins=[y_part.opt()],
                    outs=[rs_out.opt()],
                )
                nc.sync.dma_start(out=yout[:], in_=rs_out[:])

    with tile.TileContext(nc) as tc:
        with tc.tile_pool(name="res", bufs=1) as res:
            for _rep in range(reps):
                emit_body(nc, tc, res, do_coll=(_rep == reps - 1))
    nc.finalize()
    return nc


def _np_dtype(mode):
    if mode == "bf16":
        import ml_dtypes
        return ml_dtypes.bfloat16
    return np.float32


def _in_maps(query, Wq, bq, Wk, bk, Wv, bv, Wo, bo, mode):
    ndt = _np_dtype(mode)
    maps = []
    for c in range(NC):
        b, g = c // G, c % G
        gr = slice(g * GF, (g + 1) * GF)
        maps.append({
            "xT": np.ascontiguousarray(query[b].T).astype(ndt),
            "wqT": np.ascontiguousarray(Wq[gr, :].T).astype(ndt),
            "wkT": np.ascontiguousarray(Wk[gr, :].T).astype(ndt),
            "wvT": np.ascontiguousarray(Wv[gr, :].T).astype(ndt),
            "woT": np.ascontiguousarray(Wo[:, gr].T).astype(ndt),
            "bkq_c": np.ascontiguousarray(np.concatenate([
                np.asarray(bk[gr], np.float32).reshape(2, 128).T,
                np.asarray(bq[gr], np.float32).reshape(2, 128).T], axis=1)),
            "bv_b": np.ascontiguousarray(
                np.tile(np.asarray(bv[gr]).reshape(1, GF), (128, 1))
            ).astype(ndt),
        })
    return maps


def kernel(query, Wq, bq, Wk, bk, Wv, bv, Wo, bo,
           mode="bf16", collective=True, trace=False):
    from concourse.bass_utils import run_bass_kernel_spmd

    key = (mode, collective, 1)
    if key not in _CACHE:
        _CACHE[key] = _build(mode, collective)
    nc = _CACHE[key]

    maps = _in_maps(query, Wq, bq, Wk, bk, Wv, bv, Wo, bo, mode)
    res = run_bass_kernel_spmd(nc, maps, list(range(NC)), trace=trace)

    out = np.empty((B, S, E), np.float32)
    if collective:
        for c in range(NC):
            b, g = c // G, c % G
            out[b, g * (S // G):(g + 1) * (S // G), :] = np.asarray(
                res.results[c]["yout"], np.float32)
    else:
        for b in range(B):
            out[b] = sum(np.asarray(res.results[b * G + g]["yout"], np.float32)
                         for g in range(G))
    out += np.asarray(bo, np.float32)
    if trace:
        kernel.last_results = res
    return out


# revision 43
# speedup vs baseline: 1.0059x; 1.0059x over previous
"""Multi-head attention (B=2, S=2048, E=1024, H=16, D=64) on 8 trn2 cores.

Sharding: core c = (b, g) with b = c // 4 (batch), g = c % 4 (head group of
4 heads = 256 features). Each core computes Q/K/V projections for its head
group, full attention for its 4 heads, and a partial output projection
(columns of its group); a ReduceScatter over the 4 cores of each batch sums
the partials and leaves each core with a [512, 1024] slice of the final
output. The host concatenates the slices and adds bo.

Device-side layouts (host pre-transposes/casts):
  xT  [1024, 2048]  query[b].T                 (compute dtype)
  wqT/wkT/wvT [1024, 256]  W[g*256:(g+1)*256, :].T
  woT [256, 1024]          Wo[:, g*256:(g+1)*256].T
  bkq_c [128, 4]           K/Q bias columns (fp32, added in PSUM->SBUF copy)
  bv_b [128, 256]          V bias row pre-broadcast over partitions

On-chip dataflow per core (all contractions on the partition dim):
  Q^T,K^T [f,s] = (W^T chunk).T @ x^T + bias   (bias via DVE tensor_scalar)
  V [k,f] = (x^T chunk).T @ W^T + bias         (bias via DVE tensor_tensor)
  S^T [k,q] = (K^T chunk).T @ Q^T   (K = d = 64)
  P^T = exp(S^T / 8)  via ScalarE, PSUM -> SBUF, cast to compute dtype
  O  [q,d+1] = (P^T chunk).T @ V'   with V' = [V | 1] (col d = denom)
    -- flipped AV: stationary = P^T chunk, so the matmul's free dim is
       d+1 = 65 instead of 512, quartering tensor-engine time there.
  O <- O * (1/denom)  (DVE per-partition scalar multiply), then
  O^T via DMA crossbar transpose back into SBUF (idle DMA engines)
  Y [s,f] = (O^T chunk).T @ Wo^T  -> ReduceScatter(+) over the 4-core group
  (Y is written bf16; the host accumulates/adds bo in fp32.)

Scheduling: engine queues are in-order, so emission order is everything.
Dummy matmuls warm the PE p-state during the input-DMA wait; x arrives in
column groups and the K projection is split into half-tiles emitted just
ahead of the score group that first needs each one, so ScalarE starts
~10us in and then runs nearly stall-free; every iteration interleaves the
previous q-chunk's AV (split per 128-query subblock) between score groups
with a half-block skew that keeps each exp window's tensor-engine load
under budget; the last q-chunk's AV rides inside the final iteration so
the tail owes only one head + transposes + out-projection.
"""

import numpy as np

B, S, E, H, D = 2, 2048, 1024, 16, 64
G = 4            # head groups (tensor-parallel)
GH = H // G      # heads per group = 4
GF = GH * D      # features per group = 256
NC = 8
SCALE = 1.0 / np.sqrt(D)

_CACHE = {}


def _build(mode: str, collective: bool, reps: int = 1):
    import concourse.bass as bass
    import concourse.mybir as mybir
    import concourse.tile as tile
    from concourse import bacc

    dt = mybir.dt
    C = {"bf16": dt.bfloat16, "f32r": dt.float32r, "fp32": dt.float32}[mode]
    f32 = dt.float32

    nc = bacc.Bacc()

    xT = nc.dram_tensor("xT", [E, S], C, kind="ExternalInput")
    wqT = nc.dram_tensor("wqT", [E, GF], C, kind="ExternalInput")
    wkT = nc.dram_tensor("wkT", [E, GF], C, kind="ExternalInput")
    wvT = nc.dram_tensor("wvT", [E, GF], C, kind="ExternalInput")
    woT = nc.dram_tensor("woT", [GF, E], C, kind="ExternalInput")
    bkq_c = nc.dram_tensor("bkq_c", [128, 4], f32, kind="ExternalInput")
    bv_b = nc.dram_tensor("bv_b", [128, GF], C, kind="ExternalInput")
    # bf16 output halves the output DMA time; the host accumulates in fp32
    if collective:
        yout = nc.dram_tensor("yout", [S // G, E], C, kind="ExternalOutput")
    else:
        yout = nc.dram_tensor("yout", [S, E], C, kind="ExternalOutput")

    EC = E // 128    # 8 e-chunks
    QC = S // 512    # 4 q-chunks
    KB = S // 128    # 16 k-blocks
    VW = GH * (D + 1)  # 260: V' row width (per head: 64 data + 1 ones col)

    def emit_body(nc, tc, res, do_coll):
        # x as one resident tile, e-chunk major; column-group DMAs fill it
        xAll = res.tile([128, EC * S], C, tag="xAll", name="xAll")

        def xs(ec, c0, c1):
            return xAll[:, ec * S + c0:ec * S + c1]

        wqT_sb = res.tile([128, EC * GF], C, tag="wqT")
        wkT_sb = res.tile([128, EC * GF], C, tag="wkT")
        wvT_sb = res.tile([128, EC * GF], C, tag="wvT")
        woT_sb = res.tile([128, 2 * E], C, tag="woT")
        QT_sb = [[res.tile([128, 512], C, tag=f"QT{fb}_{qc}", name=f"QT{fb}_{qc}")
                  for qc in range(QC)] for fb in range(2)]
        KT_sb = [[res.tile([128, 512], C, tag=f"KT{fb}_{qc}", name=f"KT{fb}_{qc}")
                  for qc in range(QC)] for fb in range(2)]
        V_sb = [res.tile([128, VW], C, tag=f"V{kb}", name=f"V{kb}")
                for kb in range(KB)]
        # O^T per qc: [128, 2*512]: free = hb*512 + q  (hb = head-pair block)
        OT2_sb = [res.tile([128, 2 * 512], C, tag=f"OT{qc}", name=f"OT{qc}")
                  for qc in range(QC)]
        bkq_sb = res.tile([128, 4], f32, tag="bkq")
        bk_sb = bkq_sb[:, 0:2]
        bq_sb = bkq_sb[:, 2:4]
        bv_sb = res.tile([128, GF], C, tag="bv")

        # input DMAs, ordered for the critical path: wk, x cols 0:512, wq,
        # qk biases, remaining x columns, then v/o weights and constants.
        def dma_w(dst, src, nech):
            nc.sync.dma_start(
                out=dst[:].rearrange("p (g f) -> p g f", g=nech),
                in_=src[:].rearrange("(g p) f -> p g f", p=128))

        def dma_xcols(c0, c1):
            nc.sync.dma_start(
                out=xAll[:].rearrange("p (g s) -> p g s", g=EC)[:, :, c0:c1],
                in_=xT[:].rearrange("(g p) s -> p g s", p=128)[:, :, c0:c1])

        dma_w(wkT_sb, wkT, EC)
        dma_xcols(0, 256)
        dma_xcols(256, 512)
        nc.sync.dma_start(out=bkq_sb[:], in_=bkq_c[:])
        dma_w(wqT_sb, wqT, EC)
        for qp in range(1, 4):
            dma_xcols(qp * 512, qp * 512 + 512)
        nc.sync.dma_start(out=bv_sb[:], in_=bv_b[:])
        dma_w(wvT_sb, wvT, EC)
        dma_w(woT_sb, woT, 2)

        # ones columns of V'
        scrap = res.tile([1, 128], C, tag="scrap")
        nc.gpsimd.memset(scrap[:], 0.0)
        for kb in range(KB):
            nc.gpsimd.memset(
                V_sb[kb][:].rearrange("p (h x) -> p h x", x=D + 1)[:, :, D:D + 1],
                1.0)

        # ---- emit helpers ----
        def emit_qk_proj(pp, w_sb, dst, b_sb, fb, qc, c0=0, c1=512):
            # Q^T / K^T group in [f, s] layout: stationary = W^T chunk
            w = c1 - c0
            pq = pp.tile([128, 512], f32, tag="pq", name="pq", bufs=2)
            for ec in range(EC):
                nc.tensor.matmul(
                    pq[:, 0:w],
                    lhsT=w_sb[:, ec * GF + fb * 128:ec * GF + fb * 128 + 128],
                    rhs=xs(ec, qc * 512 + c0, qc * 512 + c1),
                    start=(ec == 0), stop=(ec == EC - 1))
            # bias folded into the PSUM->SBUF copy as a per-partition scalar
            nc.vector.tensor_scalar_add(
                out=dst[fb][qc][:, c0:c1], in0=pq[:, 0:w],
                scalar1=b_sb[:, fb:fb + 1])

        def emit_v_proj_pair(pp, j):
            # V group in natural [k, f] layout for k-blocks 2j, 2j+1
            pv = pp.tile([128, 512], f32, tag="pav", name="pv", bufs=2)
            for t in range(2):
                kb = 2 * j + t
                for ec in range(EC):
                    nc.tensor.matmul(
                        pv[:, t * GF:(t + 1) * GF],
                        lhsT=xs(ec, kb * 128, kb * 128 + 128),
                        rhs=wvT_sb[:, ec * GF:(ec + 1) * GF],
                        start=(ec == 0), stop=(ec == EC - 1))
            for t in range(2):
                kb = 2 * j + t
                # bias via host-broadcast row, fused into the PSUM->SBUF copy
                nc.vector.tensor_tensor(
                    out=V_sb[kb][:].rearrange(
                        "p (h x) -> p h x", x=D + 1)[:, :, 0:D],
                    in0=pv[:, t * GF:(t + 1) * GF].rearrange(
                        "p (h d) -> p h d", d=D),
                    in1=bv_sb[:].rearrange("p (h d) -> p h d", d=D),
                    op=mybir.AluOpType.add)

        def emit_score_group(ps, h, qc, ptt, gi):
            hb, hr = h // 2, (h % 2) * D
            kb0 = 2 * gi
            pst = ps.tile([128, 2 * 512], f32, tag="pst", name="pst", bufs=2)
            for kj in range(2):
                kb = kb0 + kj
                nc.tensor.matmul(
                    pst[:, kj * 512:(kj + 1) * 512],
                    lhsT=KT_sb[hb][kb // 4][hr:hr + D,
                                            (kb % 4) * 128:(kb % 4) * 128 + 128],
                    rhs=QT_sb[hb][qc][hr:hr + D, :],
                    start=True, stop=True)
            nc.scalar.activation(
                ptt[:, kb0 * 512:(kb0 + 2) * 512],
                pst[:],
                mybir.ActivationFunctionType.Exp, scale=SCALE)

        def emit_av_qs(pav, recp, O2, h, ptt, qs):
            # Flipped AV for one 128-query subblock: stationary = P^T chunk
            # [128k, 128q], moving = V' [128k, 65]; accumulate over k-blocks.
            for kb in range(KB):
                nc.tensor.matmul(
                    pav[:, qs * (D + 1):(qs + 1) * (D + 1)],
                    lhsT=ptt[:, kb * 512 + qs * 128:kb * 512 + qs * 128 + 128],
                    rhs=V_sb[kb][:, h * (D + 1):(h + 1) * (D + 1)],
                    start=(kb == 0), stop=(kb == KB - 1))
            rec = recp.tile([128, 1], f32, tag="rec", name="rec")
            nc.vector.reciprocal(
                rec[:], pav[:, qs * (D + 1) + D:qs * (D + 1) + D + 1])
            nc.vector.tensor_scalar_mul(
                out=O2[qs][:, h * D:(h + 1) * D],
                in0=pav[:, qs * (D + 1):qs * (D + 1) + D],
                scalar1=rec[:])

        def emit_transposes(pp, O2, qc, hb, eng=None):
            # O [q, f-pair] -> O^T [f-pair, q] for head pair hb, all 4 qs,
            # via the DMA crossbar transpose (idle DMA engines, no PE/DVE)
            eng = eng or nc.sync
            for qs in range(4):
                eng.dma_start_transpose(
                    out=OT2_sb[qc][:, hb * 512 + qs * 128:
                                   hb * 512 + qs * 128 + 128],
                    in_=O2[qs][:, hb * 128:hb * 128 + 128])

        def emit_outproj_sb(po, ysb, sb, act_copy=False):
            qc = sb // 4
            for fc in range(2):
                pyt = po.tile([128, 512], f32, tag=("pav" if fc == 0 else "pq"),
                              name="pyt")
                for ec in range(2):
                    nc.tensor.matmul(
                        pyt[:],
                        lhsT=OT2_sb[qc][:, ec * 512 + (sb % 4) * 128:
                                        ec * 512 + (sb % 4) * 128 + 128],
                        rhs=woT_sb[:, ec * E + fc * 512:ec * E + fc * 512 + 512],
                        start=(ec == 0), stop=(ec == 1))
                yt = ysb.tile([128, 512], C, tag="yt", name="yt")
                if act_copy and fc == 1:
                    # tail only: ScalarE is idle once the exps are done
                    nc.scalar.copy(yt[:], pyt[:])
                else:
                    nc.vector.tensor_copy(yt[:], pyt[:])
                dst = y_part if collective else yout
                nc.sync.dma_start(
                    out=dst[sb * 128:(sb + 1) * 128, fc * 512:(fc + 1) * 512],
                    in_=yt[:])

        # ---- emission (order = scheduler priority; engine queues are
        # in-order, so every dependency must appear before its consumer,
        # and slow-to-unblock work must not be emitted ahead of urgent
        # work on the same engine) ----
        # PSUM (8 banks): "pst" [128,1024] x2 = 4 banks (scores+exp),
        # "pq" [128,512] x2 = 2 banks (projections, pT staging, outproj),
        # "pav" [128,512] x2 = 2 banks (V-proj, AV accum, outproj).
        with tc.tile_pool(name="dram", bufs=1, space="DRAM") as dram, \
             tc.tile_pool(name="pall", bufs=2, space="PSUM") as pall, \
             tc.tile_pool(name="ptp", bufs=6) as ptp, \
             tc.tile_pool(name="o2p", bufs=3) as o2p, \
             tc.tile_pool(name="recp", bufs=8) as recp, \
             tc.tile_pool(name="ysb", bufs=6) as ysb:
            if collective:
                y_part = dram.tile([S, E], C, tag="ypart")
                rs_out = dram.tile([S // G, E], C, tag="rsout")

            def new_ptt(h):
                return ptp.tile([128, KB * 512], C, tag="ptt", name=f"ptt{h}")

            def exp_block(h, qc, ptt, extras):
                """Emit the 8 score groups + exp calls for (h, qc), with
                `extras` (list of thunks) interleaved between groups."""
                for gi in range(8):
                    emit_score_group(pall, h, qc, ptt, gi)
                    if gi >= 1 and extras:
                        extras.pop(0)()
                while extras:
                    extras.pop(0)()

            # --- q-chunk 0 phase: K/Q projections interleaved with the
            # first heads' score groups so ScalarE starts ASAP.
            def kp(fb, kc, half):
                return lambda: emit_qk_proj(pall, wkT_sb, KT_sb, bk_sb, fb, kc,
                                            half * 256, half * 256 + 256)

            ptts = {}
            # warm the tensor engine's p-state during the input-DMA wait:
            # back-to-back trivial matmuls keep it busy so the first real
            # projections run at full clock
            pwu = pall.tile([128, 2 * 512], f32, tag="pst", name="pwu", bufs=2)
            for _ in range(96):
                nc.tensor.matmul(pwu[0:1, 0:128], lhsT=scrap[:, 0:1],
                                 rhs=scrap[:], start=True, stop=True)
            nc.vector.tensor_copy(scrap[:], pwu[0:1, 0:128])
            # K projection in half-tiles, each emitted just ahead of the
            # score group that first needs it (in-order PE queue = JIT feed)
            kp(0, 0, 0)()
            emit_qk_proj(pall, wqT_sb, QT_sb, bq_sb, 0, 0)
            kp(0, 0, 1)()
            ptts[0] = new_ptt(0)
            exp_block(0, 0, ptts[0], [
                kp(0, 1, 0), kp(0, 1, 1), kp(0, 2, 0),
                kp(0, 2, 1), kp(0, 3, 0), kp(0, 3, 1)])
            ptts[1] = new_ptt(1)
            exp_block(1, 0, ptts[1], [
                lambda: emit_qk_proj(pall, wqT_sb, QT_sb, bq_sb, 1, 0),
                kp(1, 0, 0), kp(1, 0, 1), kp(1, 1, 0), kp(1, 1, 1)])
            ptts[2] = new_ptt(2)
            exp_block(2, 0, ptts[2], [
                kp(1, 2, 0), kp(1, 2, 1), kp(1, 3, 0), kp(1, 3, 1),
                lambda: emit_v_proj_pair(pall, 0)])
            ptts[3] = new_ptt(3)
            exp_block(3, 0, ptts[3], [
                lambda: emit_v_proj_pair(pall, 1),
                lambda: emit_qk_proj(pall, wqT_sb, QT_sb, bq_sb, 0, 1)])
            emit_qk_proj(pall, wqT_sb, QT_sb, bq_sb, 1, 1)

            # steady-state: exps of qc overlap AV of qc-1 (split per qs),
            # transposes of qc-1 complete within the iteration, outproj of
            # qc-2 rides along; the tail only owes qc3's AV + outproj.
            def av_extras(O2, h, ptt):
                # pav is allocated lazily at the first AV chunk so no other
                # same-tag allocation can slip between tile() and first write
                holder = {}

                def mk(qs):
                    def go():
                        if "pav" not in holder:
                            holder["pav"] = pall.tile(
                                [128, 512], f32, tag="pav", name="pav", bufs=2)
                        emit_av_qs(holder["pav"], recp, O2, h, ptt, qs)
                    return go
                return [mk(qs) for qs in range(4)]

            # Global half-block skew: in iteration qc, block h0 carries
            # AV(qc-2, h3) and blocks h1..h3 carry AV(qc-1, h0..h2), so
            # per-block tensor-engine load stays under the exp window.
            O2s = {}
            ptth = {0: ptts}
            for qc in range(1, QC):
                pqc = qc - 1
                O2s[pqc] = [o2p.tile([128, GH * D], C, tag=f"o2_{qs}",
                                     name="o2") for qs in range(4)]
                new_ptts = {}
                new_ptts[0] = new_ptt(0)
                if qc == 1:
                    ex = [lambda: emit_v_proj_pair(pall, 2),
                          lambda: emit_v_proj_pair(pall, 3),
                          lambda: emit_v_proj_pair(pall, 4),
                          lambda: emit_v_proj_pair(pall, 5)]
                else:
                    ex = av_extras(O2s[qc - 2], 3, ptth[qc - 2][3])
                exp_block(0, qc, new_ptts[0], ex)
                if qc >= 2:
                    emit_transposes(pall, O2s[qc - 2], qc - 2, 1)
                new_ptts[1] = new_ptt(1)
                ex = av_extras(O2s[pqc], 0, ptth[pqc][0])
                if qc == 1:
                    ex = [lambda: emit_v_proj_pair(pall, 6),
                          lambda: emit_v_proj_pair(pall, 7)] + ex
                exp_block(1, qc, new_ptts[1], ex)
                if qc >= 2:
                    for sb in range((qc - 2) * 4, (qc - 2) * 4 + 4):
                        emit_outproj_sb(pall, ysb, sb)
                new_ptts[2] = new_ptt(2)
                ex = av_extras(O2s[pqc], 1, ptth[pqc][1])
                if qc < QC - 1:
                    ex.append(lambda: emit_qk_proj(
                        pall, wqT_sb, QT_sb, bq_sb, 0, qc + 1))
                    ex.append(lambda: emit_qk_proj(
                        pall, wqT_sb, QT_sb, bq_sb, 1, qc + 1))
                else:
                    # last iteration: its own first heads' AV rides along so
                    # the tail only owes heads 2 and 3
                    O2s[qc] = [o2p.tile([128, GH * D], C, tag=f"o2_{qs}",
                                        name="o2") for qs in range(4)]
                    ex = ex + av_extras(O2s[qc], 0, new_ptts[0])
                exp_block(2, qc, new_ptts[2], ex)
                new_ptts[3] = new_ptt(3)
                ex = av_extras(O2s[pqc], 2, ptth[pqc][2])
                if qc == QC - 1:
                    ex = ex + av_extras(O2s[qc], 1, new_ptts[1])
                exp_block(3, qc, new_ptts[3], ex)
                emit_transposes(pall, O2s[pqc], pqc, 0)
                ptth[qc] = new_ptts
                ptts = new_ptts

            # tail: qc2's h3, then qc3's last heads + transposes + outproj
            q3, q2 = QC - 1, QC - 2
            for thunk in av_extras(O2s[q2], 3, ptth[q2][3]):
                thunk()
            emit_transposes(pall, O2s[q2], q2, 1)
            emit_transposes(pall, O2s[q3], q3, 0)
            for thunk in av_extras(O2s[q3], 2, ptth[q3][2]):
                thunk()
            for sb in range(q2 * 4, q2 * 4 + 4):
                emit_outproj_sb(pall, ysb, sb)
            for thunk in av_extras(O2s[q3], 3, ptth[q3][3]):
                thunk()
            emit_transposes(pall, O2s[q3], q3, 1)
            for sb in range(q3 * 4, q3 * 4 + 4):
                emit_outproj_sb(pall, ysb, sb, act_copy=True)

            if collective and do_coll:
                nc.gpsimd.collective_compute(
                    "ReduceScatter",
                    mybir.AluOpType.add,
                    replica_groups=[[0, 1, 2, 3], [4, 5, 6, 7]],
                    ins=[y_part.opt()],
                    outs=[rs_out.opt()],
                )
                nc.sync.dma_start(out=yout[:], in_=rs_out[:])

    with tile.TileContext(nc) as tc:
        with tc.tile_pool(name="res", bufs=1) as res:
            for _rep in range(reps):
                emit_body(nc, tc, res, do_coll=(_rep == reps - 1))
    nc.finalize()
    return nc


def _np_dtype(mode):
    if mode == "bf16":
        import ml_dtypes
        return ml_dtypes.bfloat16
    return np.float32


def _in_maps(query, Wq, bq, Wk, bk, Wv, bv, Wo, bo, mode):
    ndt = _np_dtype(mode)
    maps = []
    for c in range(NC):
        b, g = c // G, c % G
        gr = slice(g * GF, (g + 1) * GF)
        maps.append({
            "xT": np.ascontiguousarray(query[b].T).astype(ndt),
            "wqT": np.ascontiguousarray(Wq[gr, :].T).astype(ndt),
            "wkT": np.ascontiguousarray(Wk[gr, :].T).astype(ndt),
            "wvT": np.ascontiguousarray(Wv[gr, :].T).astype(ndt),
            "woT": np.ascontiguousarray(Wo[:, gr].T).astype(ndt),
            "bkq_c": np.ascontiguousarray(np.concatenate([
                np.asarray(bk[gr], np.float32).reshape(2, 128).T,
                np.asarray(bq[gr], np.float32).reshape(2, 128).T], axis=1)),
            "bv_b": np.ascontiguousarray(
                np.tile(np.asarray(bv[gr]).reshape(1, GF), (128, 1))
            ).astype(ndt),
        })
    return maps


def kernel(query, Wq, bq, Wk, bk, Wv, bv, Wo, bo,
           mode="bf16", collective=True, trace=False):
    from concourse.bass_utils import run_bass_kernel_spmd

    key = (mode, collective, 1)
    if key not in _CACHE:
        _CACHE[key] = _build(mode, collective)
    nc = _CACHE[key]

    maps = _in_maps(query, Wq, bq, Wk, bk, Wv, bv, Wo, bo, mode)
    res = run_bass_kernel_spmd(nc, maps, list(range(NC)), trace=trace)

    out = np.empty((B, S, E), np.float32)
    if collective:
        for c in range(NC):
            b, g = c // G, c % G
            out[b, g * (S // G):(g + 1) * (S // G), :] = np.asarray(
                res.results[c]["yout"], np.float32)
    else:
        for b in range(B):
            out[b] = sum(np.asarray(res.results[b * G + g]["yout"], np.float32)
                         for g in range(G))
    out += np.asarray(bo, np.float32)
    if trace:
        kernel.last_results = res
    return out


# revision 49
# speedup vs baseline: 1.0107x; 1.0048x over previous
"""Multi-head attention (B=2, S=2048, E=1024, H=16, D=64) on 8 trn2 cores.

Sharding: core c = (b, g) with b = c // 4 (batch), g = c % 4 (head group of
4 heads = 256 features). Each core computes Q/K/V projections for its head
group, full attention for its 4 heads, and a partial output projection
(columns of its group); a ReduceScatter over the 4 cores of each batch sums
the partials and leaves each core with a [512, 1024] slice of the final
output. The host concatenates the slices and adds bo.

Device-side layouts (host pre-transposes/casts):
  xT  [1024, 2048]  query[b].T                 (compute dtype)
  wqT/wkT/wvT [1024, 256]  W[g*256:(g+1)*256, :].T
  woT [256, 1024]          Wo[:, g*256:(g+1)*256].T
  bkq_c [128, 4]           K/Q bias columns (fp32, added in PSUM->SBUF copy)
  bv_b [128, 256]          V bias row pre-broadcast over partitions

On-chip dataflow per core (all contractions on the partition dim):
  Q^T,K^T [f,s] = (W^T chunk).T @ x^T + bias   (bias via DVE tensor_scalar)
  V [k,f] = (x^T chunk).T @ W^T + bias         (bias via DVE tensor_tensor)
  S^T [k,q] = (K^T chunk).T @ Q^T   (K = d = 64)
  P^T = exp(S^T / 8)  via ScalarE, PSUM -> SBUF, cast to compute dtype
  O  [q,d+1] = (P^T chunk).T @ V'   with V' = [V | 1] (col d = denom)
    -- flipped AV: stationary = P^T chunk, so the matmul's free dim is
       d+1 = 65 instead of 512, quartering tensor-engine time there.
  O <- O * (1/denom)  (DVE per-partition scalar multiply), then
  O^T via DMA crossbar transpose back into SBUF (idle DMA engines)
  Y [s,f] = (O^T chunk).T @ Wo^T  -> ReduceScatter(+) over the 4-core group
  (Y is written bf16; the host accumulates/adds bo in fp32.)

Scheduling: engine queues are in-order, so emission order is everything.
Dummy matmuls warm the PE p-state during the input-DMA wait; x arrives in
column groups and the K projection is split into half-tiles emitted just
ahead of the score group that first needs each one, so ScalarE starts
~10us in and then runs nearly stall-free; every iteration interleaves the
previous q-chunk's AV (split per 128-query subblock) between score groups
with a half-block skew that keeps each exp window's tensor-engine load
under budget; the last q-chunk's AV rides inside the final iteration so
the tail owes only one head + transposes + out-projection.
"""

import numpy as np

B, S, E, H, D = 2, 2048, 1024, 16, 64
G = 4            # head groups (tensor-parallel)
GH = H // G      # heads per group = 4
GF = GH * D      # features per group = 256
NC = 8
SCALE = 1.0 / np.sqrt(D)

_CACHE = {}


def _build(mode: str, collective: bool, reps: int = 1):
    import concourse.bass as bass
    import concourse.mybir as mybir
    import concourse.tile as tile
    from concourse import bacc

    dt = mybir.dt
    C = {"bf16": dt.bfloat16, "f32r": dt.float32r, "fp32": dt.float32}[mode]
    f32 = dt.float32

    nc = bacc.Bacc()

    xT = nc.dram_tensor("xT", [E, S], C, kind="ExternalInput")
    wqT = nc.dram_tensor("wqT", [E, GF], C, kind="ExternalInput")
    wkT = nc.dram_tensor("wkT", [E, GF], C, kind="ExternalInput")
    wvT = nc.dram_tensor("wvT", [E, GF], C, kind="ExternalInput")
    woT = nc.dram_tensor("woT", [GF, E], C, kind="ExternalInput")
    bkq_c = nc.dram_tensor("bkq_c", [128, 4], f32, kind="ExternalInput")
    bv_b = nc.dram_tensor("bv_b", [128, GF], C, kind="ExternalInput")
    # bf16 output halves the output DMA time; the host accumulates in fp32
    if collective:
        yout = nc.dram_tensor("yout", [S // G, E], C, kind="ExternalOutput")
    else:
        yout = nc.dram_tensor("yout", [S, E], C, kind="ExternalOutput")

    EC = E // 128    # 8 e-chunks
    QC = S // 512    # 4 q-chunks
    KB = S // 128    # 16 k-blocks
    VW = GH * (D + 1)  # 260: V' row width (per head: 64 data + 1 ones col)

    def emit_body(nc, tc, res, do_coll):
        # x as one resident tile, e-chunk major; column-group DMAs fill it
        xAll = res.tile([128, EC * S], C, tag="xAll", name="xAll")

        def xs(ec, c0, c1):
            return xAll[:, ec * S + c0:ec * S + c1]

        wqT_sb = res.tile([128, EC * GF], C, tag="wqT")
        wkT_sb = res.tile([128, EC * GF], C, tag="wkT")
        wvT_sb = res.tile([128, EC * GF], C, tag="wvT")
        woT_sb = res.tile([128, 2 * E], C, tag="woT")
        QT_sb = [[res.tile([128, 512], C, tag=f"QT{fb}_{qc}", name=f"QT{fb}_{qc}")
                  for qc in range(QC)] for fb in range(2)]
        KT_sb = [[res.tile([128, 512], C, tag=f"KT{fb}_{qc}", name=f"KT{fb}_{qc}")
                  for qc in range(QC)] for fb in range(2)]
        V_sb = [res.tile([128, VW], C, tag=f"V{kb}", name=f"V{kb}")
                for kb in range(KB)]
        # O^T per qc: [128, 2*512]: free = hb*512 + q  (hb = head-pair block)
        OT2_sb = [res.tile([128, 2 * 512], C, tag=f"OT{qc}", name=f"OT{qc}")
                  for qc in range(QC)]
        bkq_sb = res.tile([128, 4], f32, tag="bkq")
        bk_sb = bkq_sb[:, 0:2]
        bq_sb = bkq_sb[:, 2:4]
        bv_sb = res.tile([128, GF], C, tag="bv")

        # input DMAs, ordered for the critical path: wk, x cols 0:512, wq,
        # qk biases, remaining x columns, then v/o weights and constants.
        def dma_w(dst, src, nech):
            nc.sync.dma_start(
                out=dst[:].rearrange("p (g f) -> p g f", g=nech),
                in_=src[:].rearrange("(g p) f -> p g f", p=128))

        def dma_xcols(c0, c1):
            nc.sync.dma_start(
                out=xAll[:].rearrange("p (g s) -> p g s", g=EC)[:, :, c0:c1],
                in_=xT[:].rearrange("(g p) s -> p g s", p=128)[:, :, c0:c1])

        dma_w(wkT_sb, wkT, EC)
        dma_xcols(0, 256)
        dma_xcols(256, 512)
        nc.sync.dma_start(out=bkq_sb[:], in_=bkq_c[:])
        dma_w(wqT_sb, wqT, EC)
        for qp in range(1, 4):
            dma_xcols(qp * 512, qp * 512 + 512)
        nc.sync.dma_start(out=bv_sb[:], in_=bv_b[:])
        dma_w(wvT_sb, wvT, EC)
        dma_w(woT_sb, woT, 2)

        # ones columns of V'
        scrap = res.tile([1, 128], C, tag="scrap")
        nc.gpsimd.memset(scrap[:], 0.0)
        for kb in range(KB):
            nc.gpsimd.memset(
                V_sb[kb][:].rearrange("p (h x) -> p h x", x=D + 1)[:, :, D:D + 1],
                1.0)

        # ---- emit helpers ----
        def emit_qk_proj(pp, w_sb, dst, b_sb, fb, qc, c0=0, c1=512):
            # Q^T / K^T group in [f, s] layout: stationary = W^T chunk
            w = c1 - c0
            pq = pp.tile([128, 512], f32, tag="pq", name="pq", bufs=2)
            for ec in range(EC):
                nc.tensor.matmul(
                    pq[:, 0:w],
                    lhsT=w_sb[:, ec * GF + fb * 128:ec * GF + fb * 128 + 128],
                    rhs=xs(ec, qc * 512 + c0, qc * 512 + c1),
                    start=(ec == 0), stop=(ec == EC - 1))
            # bias folded into the PSUM->SBUF copy as a per-partition scalar
            nc.vector.tensor_scalar_add(
                out=dst[fb][qc][:, c0:c1], in0=pq[:, 0:w],
                scalar1=b_sb[:, fb:fb + 1])

        def emit_v_proj_pair(pp, j):
            # V group in natural [k, f] layout for k-blocks 2j, 2j+1
            pv = pp.tile([128, 512], f32, tag="pav", name="pv", bufs=2)
            for t in range(2):
                kb = 2 * j + t
                for ec in range(EC):
                    nc.tensor.matmul(
                        pv[:, t * GF:(t + 1) * GF],
                        lhsT=xs(ec, kb * 128, kb * 128 + 128),
                        rhs=wvT_sb[:, ec * GF:(ec + 1) * GF],
                        start=(ec == 0), stop=(ec == EC - 1))
            for t in range(2):
                kb = 2 * j + t
                # bias via host-broadcast row, fused into the PSUM->SBUF copy
                nc.vector.tensor_tensor(
                    out=V_sb[kb][:].rearrange(
                        "p (h x) -> p h x", x=D + 1)[:, :, 0:D],
                    in0=pv[:, t * GF:(t + 1) * GF].rearrange(
                        "p (h d) -> p h d", d=D),
                    in1=bv_sb[:].rearrange("p (h d) -> p h d", d=D),
                    op=mybir.AluOpType.add)

        def emit_score_group(ps, h, qc, ptt, gi):
            hb, hr = h // 2, (h % 2) * D
            kb0 = 2 * gi
            pst = ps.tile([128, 2 * 512], f32, tag="pst", name="pst", bufs=2)
            for kj in range(2):
                kb = kb0 + kj
                nc.tensor.matmul(
                    pst[:, kj * 512:(kj + 1) * 512],
                    lhsT=KT_sb[hb][kb // 4][hr:hr + D,
                                            (kb % 4) * 128:(kb % 4) * 128 + 128],
                    rhs=QT_sb[hb][qc][hr:hr + D, :],
                    start=True, stop=True)
            nc.scalar.activation(
                ptt[:, kb0 * 512:(kb0 + 2) * 512],
                pst[:],
                mybir.ActivationFunctionType.Exp, scale=SCALE)

        def emit_av_qs(pav, recp, O2, h, ptt, qs):
            # Flipped AV for one 128-query subblock: stationary = P^T chunk
            # [128k, 128q], moving = V' [128k, 65]; accumulate over k-blocks.
            for kb in range(KB):
                nc.tensor.matmul(
                    pav[:, qs * (D + 1):(qs + 1) * (D + 1)],
                    lhsT=ptt[:, kb * 512 + qs * 128:kb * 512 + qs * 128 + 128],
                    rhs=V_sb[kb][:, h * (D + 1):(h + 1) * (D + 1)],
                    start=(kb == 0), stop=(kb == KB - 1))
            rec = recp.tile([128, 1], f32, tag="rec", name="rec")
            nc.vector.reciprocal(
                rec[:], pav[:, qs * (D + 1) + D:qs * (D + 1) + D + 1])
            nc.vector.tensor_scalar_mul(
                out=O2[qs][:, h * D:(h + 1) * D],
                in0=pav[:, qs * (D + 1):qs * (D + 1) + D],
                scalar1=rec[:])

        def emit_transposes(pp, O2, qc, hb, eng=None):
            # O [q, f-pair] -> O^T [f-pair, q] for head pair hb, all 4 qs,
            # via the DMA crossbar transpose (idle DMA engines, no PE/DVE)
            eng = eng or nc.sync
            for qs in range(4):
                eng.dma_start_transpose(
                    out=OT2_sb[qc][:, hb * 512 + qs * 128:
                                   hb * 512 + qs * 128 + 128],
                    in_=O2[qs][:, hb * 128:hb * 128 + 128])

        def emit_outproj_sb(po, ysb, sb, act_copy=False):
            qc = sb // 4
            for fc in range(2):
                pyt = po.tile([128, 512], f32, tag=("pav" if fc == 0 else "pq"),
                              name="pyt")
                for ec in range(2):
                    nc.tensor.matmul(
                        pyt[:],
                        lhsT=OT2_sb[qc][:, ec * 512 + (sb % 4) * 128:
                                        ec * 512 + (sb % 4) * 128 + 128],
                        rhs=woT_sb[:, ec * E + fc * 512:ec * E + fc * 512 + 512],
                        start=(ec == 0), stop=(ec == 1))
                yt = ysb.tile([128, 512], C, tag="yt", name="yt")
                if act_copy and fc == 1:
                    # tail only: ScalarE is idle once the exps are done
                    nc.scalar.copy(yt[:], pyt[:])
                else:
                    nc.vector.tensor_copy(yt[:], pyt[:])
                dst = y_part if collective else yout
                nc.sync.dma_start(
                    out=dst[sb * 128:(sb + 1) * 128, fc * 512:(fc + 1) * 512],
                    in_=yt[:])

        # ---- emission (order = scheduler priority; engine queues are
        # in-order, so every dependency must appear before its consumer,
        # and slow-to-unblock work must not be emitted ahead of urgent
        # work on the same engine) ----
        # PSUM (8 banks): "pst" [128,1024] x2 = 4 banks (scores+exp),
        # "pq" [128,512] x2 = 2 banks (projections, pT staging, outproj),
        # "pav" [128,512] x2 = 2 banks (V-proj, AV accum, outproj).
        with tc.tile_pool(name="dram", bufs=1, space="DRAM") as dram, \
             tc.tile_pool(name="pall", bufs=2, space="PSUM") as pall, \
             tc.tile_pool(name="ptp", bufs=6) as ptp, \
             tc.tile_pool(name="o2p", bufs=3) as o2p, \
             tc.tile_pool(name="recp", bufs=8) as recp, \
             tc.tile_pool(name="ysb", bufs=6) as ysb:
            if collective:
                y_part = dram.tile([S, E], C, tag="ypart")
                rs_out = dram.tile([S // G, E], C, tag="rsout")

            def new_ptt(h):
                return ptp.tile([128, KB * 512], C, tag="ptt", name=f"ptt{h}")

            def exp_block(h, qc, ptt, extras):
                """Emit the 8 score groups + exp calls for (h, qc), with
                `extras` (list of thunks) interleaved between groups."""
                for gi in range(8):
                    emit_score_group(pall, h, qc, ptt, gi)
                    if gi >= 1 and extras:
                        extras.pop(0)()
                while extras:
                    extras.pop(0)()

            # --- q-chunk 0 phase: K/Q projections interleaved with the
            # first heads' score groups so ScalarE starts ASAP.
            def kp(fb, kc, half):
                return lambda: emit_qk_proj(pall, wkT_sb, KT_sb, bk_sb, fb, kc,
                                            half * 256, half * 256 + 256)

            ptts = {}
            # warm the tensor engine's p-state during the input-DMA wait:
            # back-to-back trivial matmuls keep it busy so the first real
            # projections run at full clock
            pwu = pall.tile([128, 2 * 512], f32, tag="pst", name="pwu", bufs=2)
            for _ in range(96):
                nc.tensor.matmul(pwu[0:1, 0:128], lhsT=scrap[:, 0:1],
                                 rhs=scrap[:], start=True, stop=True)
            nc.vector.tensor_copy(scrap[:], pwu[0:1, 0:128])
            # K projection in half-tiles, each emitted just ahead of the
            # score group that first needs it (in-order PE queue = JIT feed)
            kp(0, 0, 0)()
            emit_qk_proj(pall, wqT_sb, QT_sb, bq_sb, 0, 0)
            kp(0, 0, 1)()
            ptts[0] = new_ptt(0)
            exp_block(0, 0, ptts[0], [
                kp(0, 1, 0), kp(0, 1, 1), kp(0, 2, 0),
                kp(0, 2, 1), kp(0, 3, 0), kp(0, 3, 1)])
            ptts[1] = new_ptt(1)
            exp_block(1, 0, ptts[1], [
                lambda: emit_qk_proj(pall, wqT_sb, QT_sb, bq_sb, 1, 0),
                kp(1, 0, 0), kp(1, 0, 1), kp(1, 1, 0), kp(1, 1, 1)])
            ptts[2] = new_ptt(2)
            exp_block(2, 0, ptts[2], [
                kp(1, 2, 0), kp(1, 2, 1), kp(1, 3, 0), kp(1, 3, 1),
                lambda: emit_v_proj_pair(pall, 0)])
            ptts[3] = new_ptt(3)
            exp_block(3, 0, ptts[3], [
                lambda: emit_v_proj_pair(pall, 1),
                lambda: emit_qk_proj(pall, wqT_sb, QT_sb, bq_sb, 0, 1)])
            emit_qk_proj(pall, wqT_sb, QT_sb, bq_sb, 1, 1)

            # steady-state: exps of qc overlap AV of qc-1 (split per qs),
            # transposes of qc-1 complete within the iteration, outproj of
            # qc-2 rides along; the tail only owes qc3's AV + outproj.
            def av_extras(O2, h, ptt):
                # pav is allocated lazily at the first AV chunk so no other
                # same-tag allocation can slip between tile() and first write
                holder = {}

                def mk(qs):
                    def go():
                        if "pav" not in holder:
                            holder["pav"] = pall.tile(
                                [128, 512], f32, tag="pav", name="pav", bufs=2)
                        emit_av_qs(holder["pav"], recp, O2, h, ptt, qs)
                    return go
                return [mk(qs) for qs in range(4)]

            # Global half-block skew: in iteration qc, block h0 carries
            # AV(qc-2, h3) and blocks h1..h3 carry AV(qc-1, h0..h2), so
            # per-block tensor-engine load stays under the exp window.
            O2s = {}
            ptth = {0: ptts}
            for qc in range(1, QC):
                pqc = qc - 1
                O2s[pqc] = [o2p.tile([128, GH * D], C, tag=f"o2_{qs}",
                                     name="o2") for qs in range(4)]
                new_ptts = {}
                new_ptts[0] = new_ptt(0)
                if qc == 1:
                    ex = [lambda: emit_v_proj_pair(pall, 2),
                          lambda: emit_v_proj_pair(pall, 3),
                          lambda: emit_v_proj_pair(pall, 4)]
                else:
                    ex = av_extras(O2s[qc - 2], 3, ptth[qc - 2][3])
                exp_block(0, qc, new_ptts[0], ex)
                if qc >= 2:
                    emit_transposes(pall, O2s[qc - 2], qc - 2, 1)
                new_ptts[1] = new_ptt(1)
                ex = av_extras(O2s[pqc], 0, ptth[pqc][0])
                if qc == 1:
                    ex = [lambda: emit_v_proj_pair(pall, 5),
                          lambda: emit_v_proj_pair(pall, 6),
                          lambda: emit_v_proj_pair(pall, 7)] + ex
                exp_block(1, qc, new_ptts[1], ex)
                if qc >= 2:
                    for sb in range((qc - 2) * 4, (qc - 2) * 4 + 4):
                        emit_outproj_sb(pall, ysb, sb)
                new_ptts[2] = new_ptt(2)
                ex = av_extras(O2s[pqc], 1, ptth[pqc][1])
                if qc < QC - 1:
                    ex.append(lambda: emit_qk_proj(
                        pall, wqT_sb, QT_sb, bq_sb, 0, qc + 1))
                else:
                    # last iteration: its own first heads' AV rides along so
                    # the tail only owes heads 2 and 3
                    O2s[qc] = [o2p.tile([128, GH * D], C, tag=f"o2_{qs}",
                                        name="o2") for qs in range(4)]
                    ex = ex + av_extras(O2s[qc], 0, new_ptts[0])
                exp_block(2, qc, new_ptts[2], ex)
                new_ptts[3] = new_ptt(3)
                ex = av_extras(O2s[pqc], 2, ptth[pqc][2])
                if qc < QC - 1:
                    ex.append(lambda: emit_qk_proj(
                        pall, wqT_sb, QT_sb, bq_sb, 1, qc + 1))
                if qc == QC - 1:
                    ex = ex + av_extras(O2s[qc], 1, new_ptts[1])
                exp_block(3, qc, new_ptts[3], ex)
                emit_transposes(pall, O2s[pqc], pqc, 0)
                ptth[qc] = new_ptts
                ptts = new_ptts

            # tail: qc2's h3, then qc3's last heads + transposes + outproj
            q3, q2 = QC - 1, QC - 2
            for thunk in av_extras(O2s[q2], 3, ptth[q2][3]):
                thunk()
            emit_transposes(pall, O2s[q2], q2, 1)
            emit_transposes(pall, O2s[q3], q3, 0)
            for thunk in av_extras(O2s[q3], 2, ptth[q3][2]):
                thunk()
            for sb in range(q2 * 4, q2 * 4 + 4):
                emit_outproj_sb(pall, ysb, sb)
            for thunk in av_extras(O2s[q3], 3, ptth[q3][3]):
                thunk()
            emit_transposes(pall, O2s[q3], q3, 1)
            for sb in range(q3 * 4, q3 * 4 + 4):
                emit_outproj_sb(pall, ysb, sb, act_copy=True)

            if collective and do_coll:
                nc.gpsimd.collective_compute(
                    "ReduceScatter",
                    mybir.AluOpType.add,
                    replica_groups=[[0, 1, 2, 3], [4, 5, 6, 7]],
                    ins=[y_part.opt()],
                    outs=[rs_out.opt()],
                )
                nc.sync.dma_start(out=yout[:], in_=rs_out[:])

    with tile.TileContext(nc) as tc:
        with tc.tile_pool(name="res", bufs=1) as res:
            for _rep in range(reps):
                emit_body(nc, tc, res, do_coll=(_rep == reps - 1))
    nc.finalize()
    return nc


def _np_dtype(mode):
    if mode == "bf16":
        import ml_dtypes
        return ml_dtypes.bfloat16
    return np.float32


def _in_maps(query, Wq, bq, Wk, bk, Wv, bv, Wo, bo, mode):
    ndt = _np_dtype(mode)
    maps = []
    for c in range(NC):
        b, g = c // G, c % G
        gr = slice(g * GF, (g + 1) * GF)
        maps.append({
            "xT": np.ascontiguousarray(query[b].T).astype(ndt),
            "wqT": np.ascontiguousarray(Wq[gr, :].T).astype(ndt),
            "wkT": np.ascontiguousarray(Wk[gr, :].T).astype(ndt),
            "wvT": np.ascontiguousarray(Wv[gr, :].T).astype(ndt),
            "woT": np.ascontiguousarray(Wo[:, gr].T).astype(ndt),
            "bkq_c": np.ascontiguousarray(np.concatenate([
                np.asarray(bk[gr], np.float32).reshape(2, 128).T,
                np.asarray(bq[gr], np.float32).reshape(2, 128).T], axis=1)),
            "bv_b": np.ascontiguousarray(
                np.tile(np.asarray(bv[gr]).reshape(1, GF), (128, 1))
            ).astype(ndt),
        })
    return maps


def kernel(query, Wq, bq, Wk, bk, Wv, bv, Wo, bo,
           mode="bf16", collective=True, trace=False):
    from concourse.bass_utils import run_bass_kernel_spmd

    key = (mode, collective, 1)
    if key not in _CACHE:
        _CACHE[key] = _build(mode, collective)
    nc = _CACHE[key]

    maps = _in_maps(query, Wq, bq, Wk, bk, Wv, bv, Wo, bo, mode)
    res = run_bass_kernel_spmd(nc, maps, list(range(NC)), trace=trace)

    out = np.empty((B, S, E), np.float32)
    if collective:
        for c in range(NC):
            b, g = c // G, c % G
            out[b, g * (S // G):(g + 1) * (S // G), :] = np.asarray(
                res.results[c]["yout"], np.float32)
    else:
        for b in range(B):
            out[b] = sum(np.asarray(res.results[b * G + g]["yout"], np.float32)
                         for g in range(G))
    out += np.asarray(bo, np.float32)
    if trace:
        kernel.last_results = res
    return out
